# revision 1
# baseline (speedup 1.0000x reference)
"""Trainium2 Bass kernel for nn_Attention_12137577578573.

Full multi-head attention (QKV projection + masked softmax + context) for
B=4, F=T=2048, CF=CT=1024, H=16, DH=64, sharded over 8 NeuronCores as
(batch b, head-group hg): core i = (b = i // 2, hg = i % 2), each core
computing 1 batch x 8 heads.

Layout strategy (everything keyed to "contraction dim on partitions"):
  - host pre-transposes from/to tensors -> xT/yT [C, F] so the QKV
    projections contract C on partitions.
  - Q^T, K^T computed in transposed layout [cols, F]/[cols, T] so the
    scores matmul (contract DH) has DH on partitions; 2 heads are packed
    per 128-partition tile (PE row-group packing -> 2 concurrent matmuls).
  - scores come out as S^T [T, F] (T on partitions), softmax denominator
    comes for free from the context matmul via a ones-column appended to V.
  - mask folded as P = exp(alpha*S) * maskT (exp(-1e5)==0), avoiding any
    pre-exp add.
  - context: C[f,:]|d[f] = P_h^T.T @ [V_h | 1]; normalize with per-partition
    reciprocal.

The reference reshapes K as (T, DH, H) (head axis interleaved), unlike
Q/V (H, DH) — handled by a host-side column permutation of Wk/bk.
"""

import sys

if "/opt/trn_rl_repo" not in sys.path:
    sys.path.insert(0, "/opt/trn_rl_repo")

import numpy as np
import ml_dtypes

import concourse.bass as bass
import concourse.bacc as bacc
import concourse.mybir as mybir
import concourse.tile as tile
from concourse import bass_utils

BF16 = mybir.dt.bfloat16
F32 = mybir.dt.float32
bf16 = ml_dtypes.bfloat16

B, F, T, C, H, DH = 4, 2048, 2048, 1024, 16, 64
HL = 8          # heads per core
COLS = HL * DH  # 512 projected columns per core
ALPHA = 0.125   # 1/sqrt(64)
NCORES = 8
KT = C // 128   # 8 contraction tiles for projections
NFT = F // 128  # 16 F tiles
NTT = T // 128  # 16 T tiles
NFC = 4         # F chunks of 512
NPAIR = 4       # head pairs per core

# Toggled by test.py for profiling runs.
PROFILE = False
LAST_RESULTS = None

_nc_cache = None


def _emit(tc, nc, aps):
    """Software-pipelined emission.

    16 "units" (fc, pair). Unit u emits: scores+exp+mask for u, the context
    matmuls of unit u-1 (tt-outer, so they chase exp), and "filler" QKV
    projection chains woven into the score slots (V/K in u0-u1, Q(fc+1) at
    each fc's 3rd unit). This keeps PE dense (HAM warm) and starts ACT's
    exp stream ~20us in instead of after the whole QKV phase.
    """
    xT, yT, maskT, wq, wk, wv, bq, bk, bv, out = aps
    Exp = mybir.ActivationFunctionType.Exp

    import contextlib

    with contextlib.ExitStack() as ctx:
        pool = ctx.enter_context(tc.tile_pool(name="static", bufs=1))
        # rotating pools
        xTp = ctx.enter_context(tc.tile_pool(name="xTp", bufs=1))
        qTp = ctx.enter_context(tc.tile_pool(name="qTp", bufs=2))
        maskp = ctx.enter_context(tc.tile_pool(name="maskp", bufs=3))
        pTp = ctx.enter_context(tc.tile_pool(name="pTp", bufs=2))
        outp = ctx.enter_context(tc.tile_pool(name="outp", bufs=1))
        dinvp = ctx.enter_context(tc.tile_pool(name="dinvp", bufs=4))
        psum_s = ctx.enter_context(tc.tile_pool(name="psum_s", bufs=3, space="PSUM"))
        psum_ctx = ctx.enter_context(tc.tile_pool(name="psum_ctx", bufs=2, space="PSUM"))

        # Static tiles
        kT = [pool.tile([128, T], BF16, name=f"kT{cb}", tag=f"kT{cb}") for cb in range(4)]
        v = [pool.tile([128, HL * 65], BF16, name=f"v{tt}", tag=f"v{tt}") for tt in range(NTT)]
        yT_sb = [pool.tile([128, T], BF16, name=f"yT{k}", tag=f"yT{k}") for k in range(KT)]
        wq_sb = [pool.tile([128, COLS], BF16, name=f"wq{k}", tag=f"wq{k}") for k in range(KT)]
        wk_sb = [pool.tile([128, COLS], BF16, name=f"wk{k}", tag=f"wk{k}") for k in range(KT)]
        wv_sb = [pool.tile([128, COLS], BF16, name=f"wv{k}", tag=f"wv{k}") for k in range(KT)]
        bq_sb = pool.tile([128, 4], F32, name="bq_sb", tag="bq_sb")
        bk_sb = pool.tile([128, 4], F32, name="bk_sb", tag="bk_sb")
        bv_sb = pool.tile([1, COLS], BF16, name="bv_sb", tag="bv_sb")
        ones_sb = pool.tile([1, 128], BF16, name="ones_sb", tag="ones_sb")

        xT_r = xT.rearrange("(k p) f -> p k f", p=128)
        yT_r = yT.rearrange("(k p) t -> p k t", p=128)
        wq_r = wq.rearrange("(k p) c -> p k c", p=128)
        wk_r = wk.rearrange("(k p) c -> p k c", p=128)
        wv_r = wv.rearrange("(k p) c -> p k c", p=128)
        maskT_r = maskT.rearrange("(tt p) f -> p tt f", p=128)
        out_r = out.rearrange("(g p) c -> p g c", p=128)

        # ---- upfront DMA queue (sync engine, FIFO), ordered so the first
        # K/Q chains + scores can start ~15us in ----
        nc.sync.dma_start(bk_sb[:], bk[:])
        nc.sync.dma_start(bq_sb[:], bq[:])
        nc.sync.dma_start(bv_sb[:], bv[:])
        nc.vector.memset(ones_sb[:], 1.0)
        # warm up the exp table (2.7us ACT_TABLE_LOAD) during input DMA
        warm_sb = pool.tile([1, 8], F32, name="warm_sb", tag="warm_sb")
        nc.vector.memset(warm_sb[:], 0.0)
        nc.scalar.activation(warm_sb[:], warm_sb[:], Exp)

        for k in range(KT):
            nc.sync.dma_start(yT_sb[k][:], yT[k * 128:(k + 1) * 128, :])
            nc.sync.dma_start(wk_sb[k][:], wk[k * 128:(k + 1) * 128, :])
        xTt = xTp.tile([128, KT, 512], BF16, name="xTt", tag="xT")
        nc.sync.dma_start(xTt[:], xT_r[:, :, 0:512])
        for k in range(KT):
            nc.sync.dma_start(wq_sb[k][:], wq[k * 128:(k + 1) * 128, :])
        mask_h = {}
        mask_h[(0, 0)] = maskp.tile([128, 8, 512], BF16, name="mh", tag="mask")
        nc.sync.dma_start(mask_h[(0, 0)][:], maskT_r[:, 0:8, 0:512])
        mask_h[(0, 1)] = maskp.tile([128, 8, 512], BF16, name="mh", tag="mask")
        nc.sync.dma_start(mask_h[(0, 1)][:], maskT_r[:, 8:16, 0:512])
        for k in range(KT):
            nc.sync.dma_start(wv_sb[k][:], wv[k * 128:(k + 1) * 128, :])

        # ---- chain emitters (PE work units) ----
        def k_chain(cb, tcc):
            ps = psum_s.tile([128, 1024], F32, name="ps_s", tag="s")[:, 0:512]
            for k in range(KT):
                nc.tensor.matmul(
                    ps[:],
                    wk_sb[k][:, cb * 128:(cb + 1) * 128],
                    yT_sb[k][:, tcc * 512:(tcc + 1) * 512],
                    start=(k == 0),
                    stop=(k == KT - 1),
                )
            nc.vector.tensor_scalar_add(
                kT[cb][:, tcc * 512:(tcc + 1) * 512], ps[:], bk_sb[:, cb:cb + 1]
            )

        def v_chain(tt):
            ps = psum_s.tile([128, 1024], F32, name="ps_s", tag="s")[:, 0:512]
            for k in range(KT):
                nc.tensor.matmul(
                    ps[:],
                    yT_sb[k][:, tt * 128:(tt + 1) * 128],
                    wv_sb[k][:],
                    start=(k == 0),
                    stop=False,
                )
            nc.tensor.matmul(
                ps[:], ones_sb[0:1, :], bv_sb[0:1, :], start=False, stop=True
            )
            vview = v[tt].rearrange("p (h c) -> p h c", c=65)
            nc.vector.tensor_copy(
                vview[:, :, 0:64], ps.rearrange("p (h c) -> p h c", c=64)[:]
            )
            nc.vector.memset(vview[:, :, 64:65], 1.0)

        qT_tiles = {}

        def q_chain(fc, cb, xt):
            qt = qT_tiles[fc]
            ps = psum_s.tile([128, 1024], F32, name="ps_s", tag="s")[:, 0:512]
            for k in range(KT):
                nc.tensor.matmul(
                    ps[:],
                    wq_sb[k][:, cb * 128:(cb + 1) * 128],
                    xt[:, k, :],
                    start=(k == 0),
                    stop=(k == KT - 1),
                )
            nc.vector.tensor_scalar_add(
                qt[:, cb, :], ps[:], bq_sb[:, cb:cb + 1]
            )

        # ---- unit machinery ----
        pT_store = {}
        ctx_ps = {}

        def emit_scores_tt(u, tt):
            fc, pair = u // 4, u % 4
            qt = qT_tiles[fc]
            ps = psum_s.tile([128, 1024], F32, name="ps_s", tag="s")
            for hh in range(2):
                nc.tensor.matmul(
                    ps[:, hh * 512:(hh + 1) * 512],
                    kT[pair][hh * 64:(hh + 1) * 64, tt * 128:(tt + 1) * 128],
                    qt[hh * 64:(hh + 1) * 64, pair, :],
                    start=True, stop=True,
                )
            nc.scalar.activation(pT_store[u][:, tt, :], ps[:], Exp, scale=ALPHA)

        def emit_mask_4tt(u, tt0):
            fc = u // 4
            mh = mask_h[(fc, tt0 // 8)]
            o = pT_store[u][:, tt0:tt0 + 4, :].rearrange(
                "p t (h c) -> p t h c", c=512
            )
            m = mh[:, tt0 % 8: tt0 % 8 + 4, :].unsqueeze(2).broadcast_to(
                [128, 4, 2, 512]
            )
            nc.vector.tensor_mul(o[:], o[:], m)

        def emit_context_chain(cu, j, half=None):
            """Chain j in 0..7: (hh = j//4, ft = j%4), 16 sequential MMs
            accumulating one 65-col region. half=0/1 emits only tt 0-7 /
            8-15 (the same accumulation group continues across halves) so
            PE work can be spread one half-chain per slot. After each
            head's last chain, evacuate it."""
            pair = cu % 4
            hh, ft = j // 4, j % 4
            if ft == 0 and half in (None, 0):
                ctx_ps.setdefault(cu, {})[hh] = psum_ctx.tile(
                    [128, 512], F32, name="pc", tag="pc"
                )
            pc = ctx_ps[cu][hh]
            pt = pT_store[cu]
            h = pair * 2 + hh
            tts = range(NTT) if half is None else range(half * 8, half * 8 + 8)
            for tt in tts:
                nc.tensor.matmul(
                    pc[:, ft * 65:ft * 65 + 65],
                    pt[:, tt, hh * 512 + ft * 128: hh * 512 + (ft + 1) * 128],
                    v[tt][:, h * 65:(h + 1) * 65],
                    start=(tt == 0),
                    stop=(tt == NTT - 1),
                )
            if ft == 3 and half in (None, 1):
                emit_ctx_evac_h(cu, hh)

        out_tiles = {}

        def emit_ctx_evac_h(cu, hh):
            """Per-unit output staging [128, 4ft, 128cols] + per-unit DMA —
            no shared per-fc tile, so fc boundaries don't serialize on the
            previous fc's output DMA."""
            fc, pair = cu // 4, cu % 4
            if cu not in out_tiles:
                out_tiles[cu] = outp.tile(
                    [128, 4, 128], F32, name="outt", tag="out"
                )
            ot = out_tiles[cu]
            pc = ctx_ps[cu][hh]
            dinv = dinvp.tile([128, 4], F32, name="dinv", tag="dinv")
            nc.vector.reciprocal(
                dinv.rearrange("p (a b) -> p a b", b=1)[:],
                pc[:, 0:260].rearrange("p (ft c) -> p ft c", c=65)[:, :, 64:65],
            )
            for ft in range(4):
                nc.vector.tensor_scalar_mul(
                    ot[:, ft, hh * 64:(hh + 1) * 64],
                    pc[:, ft * 65: ft * 65 + 64],
                    dinv[:, ft:ft + 1],
                )
            if hh == 1:
                ctx_ps.pop(cu)
                del pT_store[cu]
                nc.gpsimd.dma_start(
                    out_r[:, fc * 4:(fc + 1) * 4, pair * 128:(pair + 1) * 128],
                    ot[:],
                )
                del out_tiles[cu]

        def unit(u, fillers, ctx_u, dmas=(), ctx_late=False):
            for d in dmas:
                d()
            pT_store[u] = pTp.tile([128, NTT, 1024], BF16, name="pT", tag="pT")
            nf = len(fillers)
            fspan = 8 if ctx_late else NTT  # fillers packed early when ctx late
            fi = 0
            for tt in range(NTT):
                emit_scores_tt(u, tt)
                if tt % 4 == 3:
                    emit_mask_4tt(u, tt - 3)
                want = nf if tt >= fspan else (tt + 1) * nf // fspan
                while fi < want:
                    fillers[fi]()
                    fi += 1
                if ctx_u is not None:
                    if ctx_late and tt >= 8:
                        emit_context_chain(ctx_u, tt - 8)
                    elif not ctx_late:
                        # one half-chain per slot: chain j's halves at
                        # slots 2j and 2j+1 -> uniform PE density
                        emit_context_chain(ctx_u, tt // 2, half=tt % 2)

        # deferred DMA emitters
        def dma_xt(fc):
            def go():
                xt = xTp.tile([128, KT, 512], BF16, name="xTt", tag="xT")
                nc.sync.dma_start(xt[:], xT_r[:, :, fc * 512:(fc + 1) * 512])
                dma_xt.tiles[fc] = xt
            return go
        dma_xt.tiles = {0: xTt}

        def dma_mask(fc, half):
            def go():
                mh = maskp.tile([128, 8, 512], BF16, name="mh", tag="mask")
                nc.sync.dma_start(
                    mh[:],
                    maskT_r[:, half * 8:(half + 1) * 8, fc * 512:(fc + 1) * 512],
                )
                mask_h[(fc, half)] = mh
            return go

        # ---- prologue: K^T(cb0) + Q^T(fc0, cb0) ----
        qT_tiles[0] = qTp.tile([128, 4, 512], BF16, name="qTt", tag="qT")
        for tcc in range(4):
            k_chain(cb=0, tcc=tcc)
        q_chain(0, 0, xTt)

        # ---- 16 units ----
        def q_fillers(fc):
            qT_tiles[fc] = qTp.tile([128, 4, 512], BF16, name="qTt", tag="qT")
            return [
                (lambda cb=cb: q_chain(fc, cb, dma_xt.tiles[fc])) for cb in range(4)
            ]

        unit(0, [lambda: q_chain(0, 1, xTt)]
                + [lambda t=t: k_chain(1, t) for t in range(4)]
                + [lambda t=t: v_chain(t) for t in range(8)],
             None, dmas=(dma_xt(1),))
        unit(1, [lambda t=t: v_chain(t) for t in range(8, 16)]
                + [lambda t=t: k_chain(2, t) for t in range(4)]
                + [lambda: q_chain(0, 2, xTt)],
             0, ctx_late=True)
        unit(2, [lambda t=t: k_chain(3, t) for t in range(4)]
                + [lambda: q_chain(0, 3, xTt)], 1)
        unit(3, q_fillers(1), 2, dmas=(dma_mask(1, 0), dma_mask(1, 1)))
        unit(4, [], 3)
        unit(5, [], 4, dmas=(dma_xt(2),))
        unit(6, [], 5)
        unit(7, q_fillers(2), 6, dmas=(dma_mask(2, 0), dma_mask(2, 1)))
        unit(8, [], 7)
        unit(9, [], 8, dmas=(dma_xt(3),))
        unit(10, [], 9)
        unit(11, q_fillers(3), 10, dmas=(dma_mask(3, 0), dma_mask(3, 1)))
        unit(12, [], 11)
        unit(13, [], 12)
        unit(14, [], 13)
        unit(15, [], 14)
        # tail: context of the last unit
        for j in range(8):
            emit_context_chain(15, j)


def _build():
    global _nc_cache
    if _nc_cache is not None:
        return _nc_cache
    nc = bacc.Bacc(
        "TRN2",
        target_bir_lowering=False,
        debug=False,
        enable_asserts=False,
        num_devices=NCORES,
    )
    xT = nc.dram_tensor("xT", [C, F], BF16, kind="ExternalInput").ap()
    yT = nc.dram_tensor("yT", [C, T], BF16, kind="ExternalInput").ap()
    maskT = nc.dram_tensor("maskT", [T, F], BF16, kind="ExternalInput").ap()
    wq = nc.dram_tensor("wq", [C, COLS], BF16, kind="ExternalInput").ap()
    wk = nc.dram_tensor("wk", [C, COLS], BF16, kind="ExternalInput").ap()
    wv = nc.dram_tensor("wv", [C, COLS], BF16, kind="ExternalInput").ap()
    bq = nc.dram_tensor("bq", [128, 4], F32, kind="ExternalInput").ap()
    bk = nc.dram_tensor("bk", [128, 4], F32, kind="ExternalInput").ap()
    bv = nc.dram_tensor("bv", [1, COLS], BF16, kind="ExternalInput").ap()
    out = nc.dram_tensor("out", [F, COLS], F32, kind="ExternalOutput").ap()

    with tile.TileContext(nc) as tc:
        _emit(tc, nc, (xT, yT, maskT, wq, wk, wv, bq, bk, bv, out))
    nc.compile()
    _nc_cache = nc
    return nc


def _kperm(hg):
    """Local K column (pair*128 + hh*64 + d) -> global Wk column d*H + h_g."""
    idx = np.empty(COLS, dtype=np.int64)
    for pair in range(NPAIR):
        for hh in range(2):
            h_g = hg * HL + pair * 2 + hh
            for d in range(DH):
                idx[pair * 128 + hh * 64 + d] = d * H + h_g
    return idx


def make_in_maps(from_tensor, to_tensor, mask, Wq, bq, Wk, bk, Wv, bv):
    per_b = {}
    for b in range(B):
        per_b[b] = (
            np.ascontiguousarray(from_tensor[b].T).astype(bf16),
            np.ascontiguousarray(to_tensor[b].T).astype(bf16),
            np.ascontiguousarray(mask[b].T).astype(bf16),
        )
    in_maps = []
    for i in range(NCORES):
        b, hg = i // 2, i % 2
        xTb, yTb, mTb = per_b[b]
        sl = slice(hg * COLS, (hg + 1) * COLS)
        kidx = _kperm(hg)
        in_maps.append(
            {
                "xT": xTb,
                "yT": yTb,
                "maskT": mTb,
                "wq": np.ascontiguousarray(Wq[:, sl]).astype(bf16),
                "wk": np.ascontiguousarray(Wk[:, kidx]).astype(bf16),
                "wv": np.ascontiguousarray(Wv[:, sl]).astype(bf16),
                "bq": np.ascontiguousarray(
                    bq[sl].astype(np.float32).reshape(4, 128).T
                ),
                "bk": np.ascontiguousarray(
                    bk[kidx].astype(np.float32).reshape(4, 128).T
                ),
                "bv": bv[sl].astype(bf16).reshape(1, COLS),
                "out": np.zeros((F, COLS), np.float32),
            }
        )
    return in_maps


def kernel(from_tensor, to_tensor, mask, Wq, bq, Wk, bk, Wv, bv):
    global LAST_RESULTS
    from_tensor = np.asarray(from_tensor, dtype=np.float32)
    to_tensor = np.asarray(to_tensor, dtype=np.float32)
    mask_np = np.asarray(mask)
    Wq = np.asarray(Wq, dtype=np.float32)
    Wk = np.asarray(Wk, dtype=np.float32)
    Wv = np.asarray(Wv, dtype=np.float32)
    bq = np.asarray(bq, dtype=np.float32)
    bk = np.asarray(bk, dtype=np.float32)
    bv = np.asarray(bv, dtype=np.float32)

    nc = _build()
    in_maps = make_in_maps(
        from_tensor, to_tensor, mask_np, Wq, bq, Wk, bk, Wv, bv
    )
    for m in in_maps:
        m.pop("out", None)
    res = bass_utils.run_bass_kernel_spmd(
        nc, in_maps, core_ids=list(range(NCORES)), trace=PROFILE
    )
    LAST_RESULTS = res
    full = np.empty((B, F, H * DH), np.float32)
    for i in range(NCORES):
        b, hg = i // 2, i % 2
        full[b, :, hg * COLS:(hg + 1) * COLS] = res.results[i]["out"]
    return full

